# revision 1
# baseline (speedup 1.0000x reference)
"""CrossSpectralAttention Trainium2 kernel.

Multi-head attention over 48x48 spatial tokens: B=2, C=256, 8 heads x
head_dim 32, N=2304 tokens. Sharded over 8 NeuronCores as 2 batches x 4
head-groups (2 heads per core). Each core computes its heads' Q/K/V
projections, attention, and a partial output projection (column slice of
Wo); the host sums the 4 partials per batch.

Math notes:
- Scores s = (q.k) * d^-0.5 lie in [-7.2, 7.2] for these inputs, so the
  softmax is computed without max-subtraction: exp via ScalarE, with the
  row-sum obtained by augmenting V with a ones column in the PV matmul
  (S^T layout keeps the softmax reduction on the PE, never across
  partitions).
- All compute is fp32.
"""

import numpy as np

import concourse.bass as bass
import concourse.tile as tile
from concourse import mybir
from concourse.bass_utils import run_bass_kernel_spmd

B = 2
C = 256
N = 2304  # 48*48
NH = 8  # total heads
HPC = 2  # heads per core
HD = 32  # head dim
GD = HPC * HD  # 64 dims per core
NC = 8  # cores
NQB = 512  # query-block size for attention
NCH = N // 128  # 18 m-chunks
SCALE = float(HD) ** -0.5

F32 = mybir.dt.float32
# float32r: fp32 storage, single-pass PE matmul (4x fp32 throughput); any
# instruction producing a matmul operand must itself write float32r.
F32R = mybir.dt.float32r

LAST_RESULTS = None  # BassKernelResults of the most recent run (for test.py)
_CACHED_NC = None


def _split_excess_waits(nc, max_waits=1):
    """This walrus build allows a single sync-wait per instruction; move
    excess waits onto same-engine NoOps inserted before the instruction."""
    state = {"uid": 0}

    def fix_block(b):
        i = 0
        insts = b.instructions
        while i < len(insts):
            inst = insts[i]
            for sub in getattr(inst, "blocks", None) or []:
                fix_block(sub)
            si = inst.sync_info
            if si is not None and si.on_wait and len(si.on_wait) > max_waits:
                waits = list(si.on_wait)
                keep, extra = waits[:max_waits], waits[max_waits:]
                inst.sync_info = mybir.SyncInfo(
                    on_wait=keep, on_update=list(si.on_update or [])
                )
                nops = []
                for j in range(0, len(extra), max_waits):
                    nop = mybir.InstNoOp(name=f"WSPLIT-{state['uid']}", ins=[], outs=[])
                    state["uid"] += 1
                    nop.engine = inst.engine
                    nop.sync_info = mybir.SyncInfo(
                        on_wait=extra[j : j + max_waits], on_update=[]
                    )
                    nops.append(nop)
                for k, nop in enumerate(nops):
                    insts.insert(i + k, nop)
                i += len(nops)
            i += 1

    for f in nc.m.functions:
        for b in f.blocks:
            fix_block(b)


def _pieces(total, piece):
    out = []
    o = 0
    while o < total:
        ln = min(piece, total - o)
        out.append((o, ln))
        o += ln
    return out


def build_nc(split=True):
    nc = bass.Bass()

    # wq_t/wk_t carry 3 replicated copies of each head's 32 W^T-columns so
    # the projection matmul writes the 3-band PE layout directly:
    # layout [C, 2 heads, 96] with cols (h, 32a+d) = W[32h+d, :].T
    x_d = nc.dram_tensor("x", [C, N], F32R, kind="ExternalInput")
    wq_d = nc.dram_tensor("wq_t", [C, HPC, 96], F32R, kind="ExternalInput")
    wk_d = nc.dram_tensor("wk_t", [C, HPC, 96], F32R, kind="ExternalInput")
    wv_d = nc.dram_tensor("wv_t", [C, GD], F32R, kind="ExternalInput")
    bq_d = nc.dram_tensor("bq", [HPC, 96, 1], F32, kind="ExternalInput")
    bk_d = nc.dram_tensor("bk", [HPC, 96, 1], F32, kind="ExternalInput")
    bv_d = nc.dram_tensor("bv", [GD, 1], F32, kind="ExternalInput")
    wo_d = nc.dram_tensor("wo_t", [GD, C], F32R, kind="ExternalInput")
    bo_d = nc.dram_tensor("bo", [C, 1], F32, kind="ExternalInput")
    out_d = nc.dram_tensor("out_t", [N, C], F32, kind="ExternalOutput")

    NG = NCH // 3  # m-chunk groups of 3 (row-band packing)

    with tile.TileContext(nc) as tc:
        with (
            tc.tile_pool(name="singles", bufs=1) as singles,
            tc.tile_pool(name="expp", bufs=3) as expp,
            tc.tile_pool(name="outp", bufs=3) as outp,
        ):
            # ---- constants / inputs to SBUF ----
            x_sb = singles.tile([128, 2, N], F32R)
            xr = x_d.rearrange("(c p) n -> p c n", p=128)
            for c in range(2):
                for off, ln in _pieces(N, 768):
                    nc.sync.dma_start(
                        out=x_sb[:, c, off : off + ln],
                        in_=xr[:, c, off : off + ln],
                    )

            w_sb = {}
            for name, d in (("q", wq_d), ("k", wk_d)):
                t = singles.tile([128, 2, HPC, 96], F32R, tag=f"w{name}")
                nc.sync.dma_start(
                    out=t, in_=d.rearrange("(c p) h d -> p c h d", p=128)
                )
                w_sb[name] = t
            wv_sb = singles.tile([128, 2, GD], F32R, tag="wv")
            nc.sync.dma_start(
                out=wv_sb, in_=wv_d.rearrange("(c p) d -> p c d", p=128)
            )
            b_sb = {}
            for name, d in (("q", bq_d), ("k", bk_d)):
                hb = []
                for h in range(HPC):
                    t = singles.tile(
                        [96, 1], F32, name=f"b{name}{h}", tag=f"b{name}{h}"
                    )
                    nc.sync.dma_start(out=t, in_=d[h, :, :])
                    hb.append(t)
                b_sb[name] = hb
            # bv replicated across partitions for the V^T layout bias add
            bv_rep = singles.tile([128, GD], F32)
            nc.sync.dma_start(
                out=bv_rep,
                in_=bass.AP(tensor=bv_d, offset=0, ap=[[0, 128], [1, GD]]),
            )
            # per-head Wo^T slices, both at partition base 0
            wo_h = []
            for h in range(HPC):
                t = singles.tile([HD, C], F32R, name=f"wo{h}", tag=f"wo{h}")
                nc.sync.dma_start(out=t, in_=wo_d[HD * h : HD * (h + 1), :])
                wo_h.append(t)
            # bo replicated across partitions: bo_rep[p, c] = bo[c]
            bo_rep = singles.tile([128, C], F32)
            nc.sync.dma_start(
                out=bo_rep,
                in_=bass.AP(tensor=bo_d, offset=0, ap=[[0, 128], [1, C]]),
            )

            # q/k in 3-band replicated layout [96, N] per head
            q_rep = [
                singles.tile([96, N], F32R, name=f"qrep{h}", tag=f"qrep{h}")
                for h in range(HPC)
            ]
            k_rep = [
                singles.tile([96, N], F32R, name=f"krep{h}", tag=f"krep{h}")
                for h in range(HPC)
            ]
            dest = {"q": q_rep, "k": k_rep}

            # ---- projections: write [96, piece] per head directly ----
            with tc.tile_pool(name="proj_psum", bufs=4, space="PSUM") as proj_psum:
                for name in ("q", "k"):
                    for h in range(HPC):
                        for off, ln in _pieces(N, 512):
                            ps = proj_psum.tile([96, 512], F32, tag="proj")
                            for c in range(2):
                                nc.tensor.matmul(
                                    ps[:, :ln],
                                    w_sb[name][:, c, h, :],
                                    x_sb[:, c, off : off + ln],
                                    start=(c == 0),
                                    stop=(c == 1),
                                )
                            nc.vector.tensor_scalar(
                                out=dest[name][h][:, off : off + ln],
                                in0=ps[:, :ln],
                                scalar1=b_sb[name][h],
                                scalar2=None,
                                op0=mybir.AluOpType.add,
                            )

            # ---- V^T directly: vhat[:, j, 33h:33h+33] = [V_t_h(chunk j) | 1]
            # V_t chunk [n=128, dv] = x_chunk^T @ Wv^T (+ bv broadcast)
            vhat = singles.tile([128, NCH, 2 * (HD + 1)], F32R)
            ones2 = singles.tile([128, 2], F32)
            nc.vector.memset(ones2, 1.0)
            with tc.tile_pool(name="tpsum", bufs=4, space="PSUM") as tpsum:
                for j in range(NCH):
                    tp = tpsum.tile([128, GD], F32, tag="vt")
                    for c in range(2):
                        nc.tensor.matmul(
                            tp,
                            x_sb[:, c, 128 * j : 128 * (j + 1)],
                            wv_sb[:, c, :],
                            start=(c == 0),
                            stop=(c == 1),
                        )
                    nc.vector.tensor_tensor(
                        out=vhat[:, j, :].rearrange("p (h c) -> p h c", h=2)[
                            :, :, :HD
                        ],
                        in0=tp.rearrange("p (h c) -> p h c", h=2),
                        in1=bv_rep.rearrange("p (h c) -> p h c", h=2),
                        op=mybir.AluOpType.add,
                    )
                    nc.vector.tensor_copy(
                        out=vhat[:, j, :].rearrange("p (h c) -> p h c", h=2)[
                            :, :, HD : HD + 1
                        ],
                        in_=ones2.rearrange("p (h c) -> p h c", h=2),
                    )

            # ---- attention + per-block tail, block-pipelined ----
            oaug_h = [
                singles.tile([HD + 1, N], F32, name=f"oaug{h}", tag=f"oaug{h}")
                for h in range(HPC)
            ]
            den_t = singles.tile([128, 2 * NCH], F32)
            inv_t = singles.tile([128, 2 * NCH], F32)
            inv_bc = [
                singles.tile([HD, N], F32, name=f"invbc{h}", tag=f"invbc{h}")
                for h in range(HPC)
            ]
            on_h = [
                singles.tile([HD, N], F32R, name=f"on{h}", tag=f"on{h}")
                for h in range(HPC)
            ]
            with (
                tc.tile_pool(name="spsum", bufs=1, space="PSUM") as spsum,
                tc.tile_pool(name="opsum", bufs=2, space="PSUM") as opsum,
                tc.tile_pool(name="wopsum", bufs=2, space="PSUM") as wopsum,
                tc.tile_pool(name="dram", bufs=1, space="DRAM") as dramp,
            ):
                den_dram = dramp.tile([2, N], F32, tag="dend")
                inv_dram = dramp.tile([2, N], F32, tag="invd")
                for qoff, qln in _pieces(N, NQB):
                    j0 = qoff // 128
                    nj = qln // 128
                    for h in range(HPC):
                        vh = slice(33 * h, 33 * h + 33)
                        o_ps = opsum.tile([HD + 1, NQB], F32, tag="o")
                        for g in range(NG):
                            s_tri = spsum.tile([128, 3 * NQB], F32, tag="s")
                            for a in range(3):
                                nc.tensor.matmul(
                                    s_tri[:, NQB * a : NQB * a + qln],
                                    k_rep[h][
                                        32 * a : 32 * a + 32,
                                        384 * g + 128 * a : 384 * g + 128 * a + 128,
                                    ],
                                    q_rep[h][
                                        32 * a : 32 * a + 32, qoff : qoff + qln
                                    ],
                                    start=True,
                                    stop=True,
                                )
                            ex = expp.tile([128, 3 * NQB], F32R, tag="ex")
                            nc.scalar.activation(
                                out=ex.rearrange("p (a c) -> p a c", a=3)[
                                    :, :, :qln
                                ],
                                in_=s_tri.rearrange("p (a c) -> p a c", a=3)[
                                    :, :, :qln
                                ],
                                func=mybir.ActivationFunctionType.Exp,
                                scale=SCALE,
                            )
                            for a in range(3):
                                nc.tensor.matmul(
                                    o_ps[:, :qln],
                                    vhat[:, 3 * g + a, vh],
                                    ex[:, NQB * a : NQB * a + qln],
                                    start=(g == 0 and a == 0),
                                    stop=(g == NG - 1 and a == 2),
                                )
                        nc.vector.tensor_copy(
                            out=oaug_h[h][:, qoff : qoff + qln],
                            in_=o_ps[:, :qln],
                        )
                        # per-block denominator -> reciprocal -> broadcast
                        # (partition<->free transposes via DRAM bounce)
                        nc.sync.dma_start(
                            out=den_dram[h : h + 1, qoff : qoff + qln],
                            in_=oaug_h[h][HD : HD + 1, qoff : qoff + qln],
                        )
                        hc = NCH * h + j0
                        nc.sync.dma_start(
                            out=den_t[:, hc : hc + nj],
                            in_=den_dram[h : h + 1, qoff : qoff + qln].rearrange(
                                "o (j p) -> (o p) j", p=128
                            ),
                        )
                        nc.vector.reciprocal(
                            out=inv_t[:, hc : hc + nj], in_=den_t[:, hc : hc + nj]
                        )
                        nc.sync.dma_start(
                            out=inv_dram[h : h + 1, qoff : qoff + qln].rearrange(
                                "o (j p) -> (o p) j", p=128
                            ),
                            in_=inv_t[:, hc : hc + nj],
                        )
                        src = inv_dram[h : h + 1, qoff : qoff + qln]
                        bc = bass.AP(
                            tensor=src.tensor,
                            offset=src.offset,
                            ap=[[0, HD]] + [list(d) for d in src.ap[1:]],
                        )
                        nc.sync.dma_start(
                            out=inv_bc[h][:, qoff : qoff + qln], in_=bc
                        )
                        nc.vector.tensor_mul(
                            out=on_h[h][:, qoff : qoff + qln],
                            in0=oaug_h[h][:HD, qoff : qoff + qln],
                            in1=inv_bc[h][:, qoff : qoff + qln],
                        )
                    # ---- output projection for this block's chunks ----
                    for j in range(j0, j0 + nj):
                        wp = wopsum.tile([128, C], F32, tag="wo")
                        for h in range(HPC):
                            nc.tensor.matmul(
                                wp,
                                on_h[h][:, 128 * j : 128 * (j + 1)],
                                wo_h[h],
                                start=(h == 0),
                                stop=(h == HPC - 1),
                            )
                        ot = outp.tile([128, C], F32, tag="ot")
                        nc.vector.tensor_add(out=ot, in0=wp, in1=bo_rep)
                        nc.sync.dma_start(
                            out=out_d[128 * j : 128 * (j + 1), :], in_=ot
                        )

    if split:
        _split_excess_waits(nc)
    return nc


def kernel(x, Wq, bq, Wk, bk, Wv, bv, Wo, bo):
    global LAST_RESULTS, _CACHED_NC
    x = np.ascontiguousarray(np.asarray(x, dtype=np.float32))
    Wq = np.asarray(Wq, dtype=np.float32)
    Wk = np.asarray(Wk, dtype=np.float32)
    Wv = np.asarray(Wv, dtype=np.float32)
    Wo = np.asarray(Wo, dtype=np.float32)
    bq = np.asarray(bq, dtype=np.float32)
    bk = np.asarray(bk, dtype=np.float32)
    bv = np.asarray(bv, dtype=np.float32)
    bo = np.asarray(bo, dtype=np.float32)

    def wrep(W, g):
        # [C, 2, 96]: head h cols = W[64g+32h : 64g+32h+32, :].T tiled 3x
        out = np.empty((C, HPC, 96), np.float32)
        for h in range(HPC):
            blk = W[GD * g + HD * h : GD * g + HD * (h + 1), :].T  # [C, 32]
            out[:, h, :] = np.tile(blk, (1, 3))
        return np.ascontiguousarray(out)

    def brep(bvec, g):
        out = np.empty((HPC, 96, 1), np.float32)
        for h in range(HPC):
            out[h, :, 0] = np.tile(bvec[GD * g + HD * h : GD * g + HD * (h + 1)], 3)
        return out

    xf = x.reshape(B, C, N)
    in_maps = []
    for core in range(NC):
        b = core // 4
        g = core % 4
        sl = slice(GD * g, GD * (g + 1))
        in_maps.append(
            {
                "x": np.ascontiguousarray(xf[b]),
                "wq_t": wrep(Wq, g),
                "wk_t": wrep(Wk, g),
                "wv_t": np.ascontiguousarray(Wv[sl, :].T),
                "bq": brep(bq, g),
                "bk": brep(bk, g),
                "bv": np.ascontiguousarray(bv[sl].reshape(GD, 1)),
                "wo_t": np.ascontiguousarray(Wo[:, sl].T),
                "bo": np.ascontiguousarray(
                    bo.reshape(C, 1) if g == 0 else np.zeros((C, 1), np.float32)
                ),
            }
        )

    if _CACHED_NC is None:
        _CACHED_NC = build_nc()
    res = run_bass_kernel_spmd(_CACHED_NC, in_maps, core_ids=list(range(NC)))
    LAST_RESULTS = res

    out = np.zeros((B, C, N), dtype=np.float32)
    for core in range(NC):
        out[core // 4] += res.results[core]["out_t"].T
    return out.reshape(B, C, 48, 48)



# revision 2
# speedup vs baseline: 1.4094x; 1.4094x over previous
"""CrossSpectralAttention Trainium2 kernel.

Multi-head attention over 48x48 spatial tokens: B=2, C=256, 8 heads x
head_dim 32, N=2304 tokens. Sharded over 8 NeuronCores as 2 batches x 4
head-groups (2 heads per core). Each core computes its heads' Q/K/V
projections, attention, and a partial output projection (column slice of
Wo); the host sums the 4 partials per batch.

Math notes:
- Scores s = (q.k) * d^-0.5 lie in [-7.2, 7.2] for these inputs, so the
  softmax is computed without max-subtraction: exp via ScalarE, with the
  row-sum obtained by augmenting V with a ones column in the PV matmul
  (S^T layout keeps the softmax reduction on the PE, never across
  partitions).
- All compute is fp32.
"""

import numpy as np

import concourse.bass as bass
import concourse.tile as tile
from concourse import mybir
from concourse.bass_utils import run_bass_kernel_spmd

B = 2
C = 256
N = 2304  # 48*48
NH = 8  # total heads
HPC = 2  # heads per core
HD = 32  # head dim
GD = HPC * HD  # 64 dims per core
NC = 8  # cores
NQB = 512  # query-block size for attention
NCH = N // 128  # 18 m-chunks
SCALE = float(HD) ** -0.5

F32 = mybir.dt.float32
# float32r: fp32 storage, single-pass PE matmul (4x fp32 throughput); any
# instruction producing a matmul operand must itself write float32r.
F32R = mybir.dt.float32r

LAST_RESULTS = None  # BassKernelResults of the most recent run (for test.py)
_CACHED_NC = None


def _split_excess_waits(nc, max_waits=1):
    """This walrus build allows a single sync-wait per instruction; move
    excess waits onto same-engine NoOps inserted before the instruction."""
    state = {"uid": 0}

    def fix_block(b):
        i = 0
        insts = b.instructions
        while i < len(insts):
            inst = insts[i]
            for sub in getattr(inst, "blocks", None) or []:
                fix_block(sub)
            si = inst.sync_info
            if si is not None and si.on_wait and len(si.on_wait) > max_waits:
                waits = list(si.on_wait)
                keep, extra = waits[:max_waits], waits[max_waits:]
                inst.sync_info = mybir.SyncInfo(
                    on_wait=keep, on_update=list(si.on_update or [])
                )
                nops = []
                for j in range(0, len(extra), max_waits):
                    nop = mybir.InstNoOp(name=f"WSPLIT-{state['uid']}", ins=[], outs=[])
                    state["uid"] += 1
                    nop.engine = inst.engine
                    nop.sync_info = mybir.SyncInfo(
                        on_wait=extra[j : j + max_waits], on_update=[]
                    )
                    nops.append(nop)
                for k, nop in enumerate(nops):
                    insts.insert(i + k, nop)
                i += len(nops)
            i += 1

    for f in nc.m.functions:
        for b in f.blocks:
            fix_block(b)


def _pieces(total, piece):
    out = []
    o = 0
    while o < total:
        ln = min(piece, total - o)
        out.append((o, ln))
        o += ln
    return out


def build_nc(split=True):
    nc = bass.Bass()

    # wq_t/wk_t carry 3 replicated copies of each head's 32 W^T-columns so
    # the projection matmul writes the 3-band PE layout directly:
    # layout [C, 2 heads, 96] with cols (h, 32a+d) = W[32h+d, :].T
    x_d = nc.dram_tensor("x", [C, N], F32R, kind="ExternalInput")
    wq_d = nc.dram_tensor("wq_t", [C, HPC, 96], F32R, kind="ExternalInput")
    wk_d = nc.dram_tensor("wk_t", [C, HPC, 96], F32R, kind="ExternalInput")
    wv_d = nc.dram_tensor("wv_t", [C, GD], F32R, kind="ExternalInput")
    bq_d = nc.dram_tensor("bq", [HPC, 96, 1], F32, kind="ExternalInput")
    bk_d = nc.dram_tensor("bk", [HPC, 96, 1], F32, kind="ExternalInput")
    bv_d = nc.dram_tensor("bv", [GD, 1], F32, kind="ExternalInput")
    wo_d = nc.dram_tensor("wo_t", [GD, C], F32R, kind="ExternalInput")
    bo_d = nc.dram_tensor("bo", [C, 1], F32, kind="ExternalInput")
    out_d = nc.dram_tensor("out_t", [N, C], F32, kind="ExternalOutput")

    NG = NCH // 3  # m-chunk groups of 3 (row-band packing)

    with tile.TileContext(nc) as tc:
        with (
            tc.tile_pool(name="singles", bufs=1) as singles,
            tc.tile_pool(name="expp", bufs=3) as expp,
            tc.tile_pool(name="outp", bufs=3) as outp,
        ):
            # ---- constants / inputs to SBUF ----
            x_sb = singles.tile([128, 2, N], F32R)
            xr = x_d.rearrange("(c p) n -> p c n", p=128)
            for c in range(2):
                for off, ln in _pieces(N, 768):
                    nc.sync.dma_start(
                        out=x_sb[:, c, off : off + ln],
                        in_=xr[:, c, off : off + ln],
                    )

            w_sb = {}
            for name, d in (("q", wq_d), ("k", wk_d)):
                t = singles.tile([128, 2, HPC, 96], F32R, tag=f"w{name}")
                nc.sync.dma_start(
                    out=t, in_=d.rearrange("(c p) h d -> p c h d", p=128)
                )
                w_sb[name] = t
            wv_sb = singles.tile([128, 2, GD], F32R, tag="wv")
            nc.sync.dma_start(
                out=wv_sb, in_=wv_d.rearrange("(c p) d -> p c d", p=128)
            )
            b_sb = {}
            for name, d in (("q", bq_d), ("k", bk_d)):
                hb = []
                for h in range(HPC):
                    t = singles.tile(
                        [96, 1], F32, name=f"b{name}{h}", tag=f"b{name}{h}"
                    )
                    nc.sync.dma_start(out=t, in_=d[h, :, :])
                    hb.append(t)
                b_sb[name] = hb
            # bv replicated across partitions for the V^T layout bias add
            bv_rep = singles.tile([128, GD], F32)
            nc.sync.dma_start(
                out=bv_rep,
                in_=bass.AP(tensor=bv_d, offset=0, ap=[[0, 128], [1, GD]]),
            )
            # per-head Wo^T slices, both at partition base 0
            wo_h = []
            for h in range(HPC):
                t = singles.tile([HD, C], F32R, name=f"wo{h}", tag=f"wo{h}")
                nc.sync.dma_start(out=t, in_=wo_d[HD * h : HD * (h + 1), :])
                wo_h.append(t)
            # bo replicated across partitions: bo_rep[p, c] = bo[c]
            bo_rep = singles.tile([128, C], F32)
            nc.sync.dma_start(
                out=bo_rep,
                in_=bass.AP(tensor=bo_d, offset=0, ap=[[0, 128], [1, C]]),
            )

            # q/k in 3-band replicated layout [96, N] per head
            q_rep = [
                singles.tile([96, N], F32R, name=f"qrep{h}", tag=f"qrep{h}")
                for h in range(HPC)
            ]
            k_rep = [
                singles.tile([96, N], F32R, name=f"krep{h}", tag=f"krep{h}")
                for h in range(HPC)
            ]
            dest = {"q": q_rep, "k": k_rep}

            # ---- projections: write [96, piece] per head directly ----
            with tc.tile_pool(name="proj_psum", bufs=4, space="PSUM") as proj_psum:
                for name in ("q", "k"):
                    for h in range(HPC):
                        for off, ln in _pieces(N, 512):
                            ps = proj_psum.tile([96, 512], F32, tag="proj")
                            for c in range(2):
                                nc.tensor.matmul(
                                    ps[:, :ln],
                                    w_sb[name][:, c, h, :],
                                    x_sb[:, c, off : off + ln],
                                    start=(c == 0),
                                    stop=(c == 1),
                                )
                            nc.vector.tensor_scalar(
                                out=dest[name][h][:, off : off + ln],
                                in0=ps[:, :ln],
                                scalar1=b_sb[name][h],
                                scalar2=None,
                                op0=mybir.AluOpType.add,
                            )

            # ---- V^T directly: vhat[:, j, 33h:33h+33] = [V_t_h(chunk j) | 1]
            # V_t chunk [n=128, dv] = x_chunk^T @ Wv^T (+ bv broadcast)
            vhat = singles.tile([128, NCH, 2 * (HD + 1)], F32R)
            ones2 = singles.tile([128, 2], F32)
            nc.vector.memset(ones2, 1.0)
            with tc.tile_pool(name="tpsum", bufs=4, space="PSUM") as tpsum:
                for j in range(NCH):
                    tp = tpsum.tile([128, GD], F32, tag="vt")
                    for c in range(2):
                        nc.tensor.matmul(
                            tp,
                            x_sb[:, c, 128 * j : 128 * (j + 1)],
                            wv_sb[:, c, :],
                            start=(c == 0),
                            stop=(c == 1),
                        )
                    nc.vector.tensor_tensor(
                        out=vhat[:, j, :].rearrange("p (h c) -> p h c", h=2)[
                            :, :, :HD
                        ],
                        in0=tp.rearrange("p (h c) -> p h c", h=2),
                        in1=bv_rep.rearrange("p (h c) -> p h c", h=2),
                        op=mybir.AluOpType.add,
                    )
                    nc.vector.tensor_copy(
                        out=vhat[:, j, :].rearrange("p (h c) -> p h c", h=2)[
                            :, :, HD : HD + 1
                        ],
                        in_=ones2.rearrange("p (h c) -> p h c", h=2),
                    )

            # ---- attention + per-block tail, block-pipelined ----
            oaug_h = [
                singles.tile([HD + 1, N], F32, name=f"oaug{h}", tag=f"oaug{h}")
                for h in range(HPC)
            ]
            den_t = singles.tile([128, 2 * NCH], F32)
            inv_t = singles.tile([128, 2 * NCH], F32)
            inv_bc = [
                singles.tile([HD, N], F32, name=f"invbc{h}", tag=f"invbc{h}")
                for h in range(HPC)
            ]
            on_h = [
                singles.tile([HD, N], F32R, name=f"on{h}", tag=f"on{h}")
                for h in range(HPC)
            ]
            # PSUM budget (8 banks x 2KB): spsum 2x3 banks + opsum 1 + wopsum 1.
            # Double-buffered S tiles let QK of group g+1 overlap exp of group
            # g, keeping the PE continuously busy (it also ramps to the 2.4GHz
            # p-state only after ~3us of uninterrupted work).
            with (
                tc.tile_pool(name="spsum", bufs=2, space="PSUM") as spsum,
                tc.tile_pool(name="opsum", bufs=1, space="PSUM") as opsum,
                tc.tile_pool(name="wopsum", bufs=1, space="PSUM") as wopsum,
                tc.tile_pool(name="dram", bufs=1, space="DRAM") as dramp,
            ):
                den_dram = dramp.tile([2, N], F32, tag="dend")
                inv_dram = dramp.tile([2, N], F32, tag="invd")
                for qoff, qln in _pieces(N, NQB):
                    j0 = qoff // 128
                    nj = qln // 128
                    for h in range(HPC):
                        vh = slice(33 * h, 33 * h + 33)
                        o_ps = opsum.tile([HD + 1, NQB], F32, tag="o")
                        for g in range(NG):
                            s_tri = spsum.tile([128, 3 * NQB], F32, tag="s")
                            for a in range(3):
                                nc.tensor.matmul(
                                    s_tri[:, NQB * a : NQB * a + qln],
                                    k_rep[h][
                                        32 * a : 32 * a + 32,
                                        384 * g + 128 * a : 384 * g + 128 * a + 128,
                                    ],
                                    q_rep[h][
                                        32 * a : 32 * a + 32, qoff : qoff + qln
                                    ],
                                    start=True,
                                    stop=True,
                                )
                            ex = expp.tile([128, 3 * NQB], F32R, tag="ex")
                            nc.scalar.activation(
                                out=ex.rearrange("p (a c) -> p a c", a=3)[
                                    :, :, :qln
                                ],
                                in_=s_tri.rearrange("p (a c) -> p a c", a=3)[
                                    :, :, :qln
                                ],
                                func=mybir.ActivationFunctionType.Exp,
                                scale=SCALE,
                            )
                            for a in range(3):
                                nc.tensor.matmul(
                                    o_ps[:, :qln],
                                    vhat[:, 3 * g + a, vh],
                                    ex[:, NQB * a : NQB * a + qln],
                                    start=(g == 0 and a == 0),
                                    stop=(g == NG - 1 and a == 2),
                                )
                        nc.vector.tensor_copy(
                            out=oaug_h[h][:, qoff : qoff + qln],
                            in_=o_ps[:, :qln],
                        )
                        # per-block denominator -> reciprocal -> broadcast
                        # (partition<->free transposes via DRAM bounce)
                        nc.sync.dma_start(
                            out=den_dram[h : h + 1, qoff : qoff + qln],
                            in_=oaug_h[h][HD : HD + 1, qoff : qoff + qln],
                        )
                        hc = NCH * h + j0
                        nc.sync.dma_start(
                            out=den_t[:, hc : hc + nj],
                            in_=den_dram[h : h + 1, qoff : qoff + qln].rearrange(
                                "o (j p) -> (o p) j", p=128
                            ),
                        )
                        nc.vector.reciprocal(
                            out=inv_t[:, hc : hc + nj], in_=den_t[:, hc : hc + nj]
                        )
                        nc.sync.dma_start(
                            out=inv_dram[h : h + 1, qoff : qoff + qln].rearrange(
                                "o (j p) -> (o p) j", p=128
                            ),
                            in_=inv_t[:, hc : hc + nj],
                        )
                        src = inv_dram[h : h + 1, qoff : qoff + qln]
                        bc = bass.AP(
                            tensor=src.tensor,
                            offset=src.offset,
                            ap=[[0, HD]] + [list(d) for d in src.ap[1:]],
                        )
                        nc.sync.dma_start(
                            out=inv_bc[h][:, qoff : qoff + qln], in_=bc
                        )
                        nc.vector.tensor_mul(
                            out=on_h[h][:, qoff : qoff + qln],
                            in0=oaug_h[h][:HD, qoff : qoff + qln],
                            in1=inv_bc[h][:, qoff : qoff + qln],
                        )
                    # ---- output projection for this block's chunks ----
                    for j in range(j0, j0 + nj):
                        wp = wopsum.tile([128, C], F32, tag="wo")
                        for h in range(HPC):
                            nc.tensor.matmul(
                                wp,
                                on_h[h][:, 128 * j : 128 * (j + 1)],
                                wo_h[h],
                                start=(h == 0),
                                stop=(h == HPC - 1),
                            )
                        ot = outp.tile([128, C], F32, tag="ot")
                        nc.vector.tensor_add(out=ot, in0=wp, in1=bo_rep)
                        nc.sync.dma_start(
                            out=out_d[128 * j : 128 * (j + 1), :], in_=ot
                        )

    if split:
        _split_excess_waits(nc)
    return nc


def kernel(x, Wq, bq, Wk, bk, Wv, bv, Wo, bo):
    global LAST_RESULTS, _CACHED_NC
    x = np.ascontiguousarray(np.asarray(x, dtype=np.float32))
    Wq = np.asarray(Wq, dtype=np.float32)
    Wk = np.asarray(Wk, dtype=np.float32)
    Wv = np.asarray(Wv, dtype=np.float32)
    Wo = np.asarray(Wo, dtype=np.float32)
    bq = np.asarray(bq, dtype=np.float32)
    bk = np.asarray(bk, dtype=np.float32)
    bv = np.asarray(bv, dtype=np.float32)
    bo = np.asarray(bo, dtype=np.float32)

    def wrep(W, g):
        # [C, 2, 96]: head h cols = W[64g+32h : 64g+32h+32, :].T tiled 3x
        out = np.empty((C, HPC, 96), np.float32)
        for h in range(HPC):
            blk = W[GD * g + HD * h : GD * g + HD * (h + 1), :].T  # [C, 32]
            out[:, h, :] = np.tile(blk, (1, 3))
        return np.ascontiguousarray(out)

    def brep(bvec, g):
        out = np.empty((HPC, 96, 1), np.float32)
        for h in range(HPC):
            out[h, :, 0] = np.tile(bvec[GD * g + HD * h : GD * g + HD * (h + 1)], 3)
        return out

    xf = x.reshape(B, C, N)
    in_maps = []
    for core in range(NC):
        b = core // 4
        g = core % 4
        sl = slice(GD * g, GD * (g + 1))
        in_maps.append(
            {
                "x": np.ascontiguousarray(xf[b]),
                "wq_t": wrep(Wq, g),
                "wk_t": wrep(Wk, g),
                "wv_t": np.ascontiguousarray(Wv[sl, :].T),
                "bq": brep(bq, g),
                "bk": brep(bk, g),
                "bv": np.ascontiguousarray(bv[sl].reshape(GD, 1)),
                "wo_t": np.ascontiguousarray(Wo[:, sl].T),
                "bo": np.ascontiguousarray(
                    bo.reshape(C, 1) if g == 0 else np.zeros((C, 1), np.float32)
                ),
            }
        )

    if _CACHED_NC is None:
        _CACHED_NC = build_nc()
    res = run_bass_kernel_spmd(_CACHED_NC, in_maps, core_ids=list(range(NC)))
    LAST_RESULTS = res

    out = np.zeros((B, C, N), dtype=np.float32)
    for core in range(NC):
        out[core // 4] += res.results[core]["out_t"].T
    return out.reshape(B, C, 48, 48)



# revision 9
# speedup vs baseline: 1.8904x; 1.3413x over previous
"""CrossSpectralAttention Trainium2 kernel.

Multi-head attention over 48x48 spatial tokens: B=2, C=256, 8 heads x
head_dim 32, N=2304 tokens. Sharded over 8 NeuronCores as 2 batches x 4
head-groups (2 heads per core). Each core computes its heads' Q/K/V
projections, attention, and a partial output projection (column slice of
Wo); the host sums the 4 partials per batch.

Math notes:
- Scores s = (q.k) * d^-0.5 lie in [-7.2, 7.2] for these inputs, so the
  softmax is computed without max-subtraction: exp via ScalarE, with the
  row-sum obtained by augmenting V with a ones column in the PV matmul
  (S^T layout keeps the softmax reduction on the PE, never across
  partitions).
- All compute is fp32.

Schedule notes:
- The kernel is emitted in software-pipelined order: per stage (one
  3-chunk key group), the QK score matmuls and the exp go out first and
  the PV matmuls of the *previous* stage follow, so the PE always has
  ready work while the ScalarE exponentiates (engines execute their
  queues in program order). The S-score PSUM is double buffered.
- Stage order is head-major; the second head's Q/K projections are
  interleaved into the first head's attention stages, and the Wo output
  projection for query block b is deferred until deep into block b+1 so
  the denominator DMA round-trip never stalls the PE.
"""

from collections import deque

import numpy as np

import concourse.bass as bass
import concourse.tile as tile
from concourse import mybir
from concourse.bass_utils import run_bass_kernel_spmd

B = 2
C = 256
N = 2304  # 48*48
NH = 8  # total heads
HPC = 2  # heads per core
HD = 32  # head dim
GD = HPC * HD  # 64 dims per core
NC = 8  # cores
NQB = 512  # query-block size for attention
NCH = N // 128  # 18 m-chunks
NG = NCH // 3  # 6 key groups of 3 chunks per stage
SCALE = float(HD) ** -0.5

F32 = mybir.dt.float32
# float32r: fp32 storage, single-pass PE matmul (4x fp32 throughput); any
# instruction producing a matmul operand must itself write float32r.
F32R = mybir.dt.float32r

LAST_RESULTS = None  # BassKernelResults of the most recent run (for test.py)
_CACHED_NC = None


def _split_excess_waits(nc, max_waits=1):
    """This walrus build allows a single sync-wait per instruction; move
    excess waits onto same-engine NoOps inserted before the instruction."""
    state = {"uid": 0}

    def fix_block(b):
        i = 0
        insts = b.instructions
        while i < len(insts):
            inst = insts[i]
            for sub in getattr(inst, "blocks", None) or []:
                fix_block(sub)
            si = inst.sync_info
            if si is not None and si.on_wait and len(si.on_wait) > max_waits:
                waits = list(si.on_wait)
                keep, extra = waits[:max_waits], waits[max_waits:]
                inst.sync_info = mybir.SyncInfo(
                    on_wait=keep, on_update=list(si.on_update or [])
                )
                nops = []
                for j in range(0, len(extra), max_waits):
                    nop = mybir.InstNoOp(name=f"WSPLIT-{state['uid']}", ins=[], outs=[])
                    state["uid"] += 1
                    nop.engine = inst.engine
                    nop.sync_info = mybir.SyncInfo(
                        on_wait=extra[j : j + max_waits], on_update=[]
                    )
                    nops.append(nop)
                for k, nop in enumerate(nops):
                    insts.insert(i + k, nop)
                i += len(nops)
            i += 1

    for f in nc.m.functions:
        for b in f.blocks:
            fix_block(b)


def _pieces(total, piece):
    out = []
    o = 0
    while o < total:
        ln = min(piece, total - o)
        out.append((o, ln))
        o += ln
    return out


def build_nc(split=True):
    nc = bass.Bass()

    # wq_t/wk_t carry 3 replicated copies of each head's 32 W^T-columns so
    # the projection matmul writes the 3-band PE layout directly:
    # layout [C, 2 heads, 96] with cols (h, 32a+d) = W[32h+d, :].T
    x_d = nc.dram_tensor("x", [C, N], F32R, kind="ExternalInput")
    wq_d = nc.dram_tensor("wq_t", [C, HPC, 96], F32R, kind="ExternalInput")
    wk_d = nc.dram_tensor("wk_t", [C, HPC, 96], F32R, kind="ExternalInput")
    wv_d = nc.dram_tensor("wv_t", [C, GD], F32R, kind="ExternalInput")
    bq_d = nc.dram_tensor("bq", [HPC, 96, 1], F32, kind="ExternalInput")
    bk_d = nc.dram_tensor("bk", [HPC, 96, 1], F32, kind="ExternalInput")
    bv_d = nc.dram_tensor("bv", [GD, 1], F32, kind="ExternalInput")
    wo_d = nc.dram_tensor("wo_t", [GD, C], F32R, kind="ExternalInput")
    bo_d = nc.dram_tensor("bo", [C, 1], F32, kind="ExternalInput")
    out_d = nc.dram_tensor("out_t", [N, C], F32, kind="ExternalOutput")

    qpieces = _pieces(N, NQB)

    with tile.TileContext(nc) as tc:
        with (
            tc.tile_pool(name="singles", bufs=1) as singles,
            tc.tile_pool(name="expp", bufs=3) as expp,
            tc.tile_pool(name="outp", bufs=3) as outp,
        ):
            # ---- input DMAs: wk/wv first (earliest consumers), then x ----
            w_sb = {}
            for name, d in (("k", wk_d), ("q", wq_d)):
                t = singles.tile([128, 2, HPC, 96], F32R, tag=f"w{name}")
                w_sb[name] = t
            wv_sb = singles.tile([128, 2, GD], F32R, tag="wv")
            x_sb = singles.tile([128, 2, N], F32R)

            nc.sync.dma_start(
                out=w_sb["k"], in_=wk_d.rearrange("(c p) h d -> p c h d", p=128)
            )
            nc.sync.dma_start(
                out=wv_sb, in_=wv_d.rearrange("(c p) d -> p c d", p=128)
            )
            xr = x_d.rearrange("(c p) n -> p c n", p=128)
            for off, ln in _pieces(N, NQB):
                for c in range(2):
                    nc.sync.dma_start(
                        out=x_sb[:, c, off : off + ln],
                        in_=xr[:, c, off : off + ln],
                    )
            nc.sync.dma_start(
                out=w_sb["q"], in_=wq_d.rearrange("(c p) h d -> p c h d", p=128)
            )

            b_sb = {}
            for name, d in (("k", bk_d), ("q", bq_d)):
                hb = []
                for h in range(HPC):
                    t = singles.tile(
                        [96, 1], F32, name=f"b{name}{h}", tag=f"b{name}{h}"
                    )
                    nc.sync.dma_start(out=t, in_=d[h, :, :])
                    hb.append(t)
                b_sb[name] = hb
            # bv replicated across partitions for the V^T layout bias add
            bv_rep = singles.tile([128, GD], F32)
            nc.sync.dma_start(
                out=bv_rep,
                in_=bass.AP(tensor=bv_d, offset=0, ap=[[0, 128], [1, GD]]),
            )
            # per-head Wo^T slices, both at partition base 0
            wo_h = []
            for h in range(HPC):
                t = singles.tile([HD, C], F32R, name=f"wo{h}", tag=f"wo{h}")
                nc.sync.dma_start(out=t, in_=wo_d[HD * h : HD * (h + 1), :])
                wo_h.append(t)
            # bo replicated across partitions: bo_rep[p, c] = bo[c]
            bo_rep = singles.tile([128, C], F32)
            nc.sync.dma_start(
                out=bo_rep,
                in_=bass.AP(tensor=bo_d, offset=0, ap=[[0, 128], [1, C]]),
            )

            # ---- persistent SBUF state ----
            q_rep = [
                singles.tile([96, N], F32R, name=f"qrep{h}", tag=f"qrep{h}")
                for h in range(HPC)
            ]
            k_rep = [
                singles.tile([96, N], F32R, name=f"krep{h}", tag=f"krep{h}")
                for h in range(HPC)
            ]
            dest = {"q": q_rep, "k": k_rep}
            vhat = singles.tile([128, NCH, 2 * (HD + 1)], F32R)
            ones2 = singles.tile([128, 2], F32)
            nc.vector.memset(ones2, 1.0)
            oaug_h = [
                singles.tile([HD + 1, N], F32, name=f"oaug{h}", tag=f"oaug{h}")
                for h in range(HPC)
            ]
            # reciprocal of the denominator row stays on partition HD (same
            # partition in and out); heads use disjoint column ranges
            inv_row = singles.tile([HD + 1, HPC * N], F32)
            inv_bc = [
                singles.tile([HD, N], F32, name=f"invbc{h}", tag=f"invbc{h}")
                for h in range(HPC)
            ]
            on_h = [
                singles.tile([HD, N], F32R, name=f"on{h}", tag=f"on{h}")
                for h in range(HPC)
            ]

            # ---- prefix projections: k head0 (all pieces), V^T (all
            # chunks), q head0 piece0 — the minimum needed to start
            # attention.  The rest is interleaved into attention stages.
            with tc.tile_pool(name="prefp", bufs=4, space="PSUM") as prefp:

                def emit_qk_proj(pool, name, h, off, ln, tag):
                    ps = pool.tile([96, NQB], F32, tag=tag, name="projps")
                    for c in range(2):
                        nc.tensor.matmul(
                            ps[:, :ln],
                            w_sb[name][:, c, h, :],
                            x_sb[:, c, off : off + ln],
                            start=(c == 0),
                            stop=(c == 1),
                        )
                    nc.vector.tensor_scalar(
                        out=dest[name][h][:, off : off + ln],
                        in0=ps[:, :ln],
                        scalar1=b_sb[name][h],
                        scalar2=None,
                        op0=mybir.AluOpType.add,
                    )

                def emit_vt(pool, j, tag):
                    # vhat[:, j, 33h:33h+33] = [V_t_h(chunk j) | 1]
                    tp = pool.tile([128, GD], F32, tag=tag, name="vtps")
                    for c in range(2):
                        nc.tensor.matmul(
                            tp,
                            x_sb[:, c, 128 * j : 128 * (j + 1)],
                            wv_sb[:, c, :],
                            start=(c == 0),
                            stop=(c == 1),
                        )
                    nc.vector.tensor_tensor(
                        out=vhat[:, j, :].rearrange("p (h c) -> p h c", h=2)[
                            :, :, :HD
                        ],
                        in0=tp.rearrange("p (h c) -> p h c", h=2),
                        in1=bv_rep.rearrange("p (h c) -> p h c", h=2),
                        op=mybir.AluOpType.add,
                    )
                    nc.vector.tensor_copy(
                        out=vhat[:, j, :].rearrange("p (h c) -> p h c", h=2)[
                            :, :, HD : HD + 1
                        ],
                        in_=ones2.rearrange("p (h c) -> p h c", h=2),
                    )

                for pi, (off, ln) in enumerate(qpieces):
                    emit_qk_proj(prefp, "k", 0, off, ln, tag="pp")
                    for j in range(4 * pi, min(4 * pi + 4, NCH)):
                        emit_vt(prefp, j, tag="vt")
                for j in range(4 * len(qpieces), NCH):
                    emit_vt(prefp, j, tag="vt")
                emit_qk_proj(prefp, "q", 0, 0, NQB, tag="pp")

            # remaining projections, interleaved into attention stages
            # (ordered: q0 rest before k1/q1 — q0 piece b is needed when
            # head0 reaches query block b; head1 needs k1/q1 at stage 30)
            projq = deque()
            for off, ln in qpieces[1:]:
                projq.append(("q", 0, off, ln))
            for off, ln in qpieces:
                projq.append(("k", 1, off, ln))
            for off, ln in qpieces:
                projq.append(("q", 1, off, ln))

            # ---- attention: software-pipelined stages ----
            # PSUM: spsum 2x3 banks + opsum 1 + wopsum 1 = 8 banks.  The
            # wopsum slot is shared (same tag) between interleaved
            # projection pieces and Wo output chunks.
            stage_list = []
            for h in range(HPC):
                for qoff, qln in qpieces:
                    for g in range(NG):
                        stage_list.append((h, qoff, qln, g))

            with (
                tc.tile_pool(name="spsum", bufs=2, space="PSUM") as spsum,
                tc.tile_pool(name="opsum", bufs=1, space="PSUM") as opsum,
                tc.tile_pool(name="wopsum", bufs=1, space="PSUM") as wopsum,
                tc.tile_pool(name="dram", bufs=1, space="DRAM") as dramp,
            ):
                inv_dram = dramp.tile([HPC, N], F32, tag="invd")
                state = {"o_ps": None}

                def flush_pv(p):
                    if p is None:
                        return
                    ex, h, qoff, qln, g = p
                    vh = slice(33 * h, 33 * h + 33)
                    if g == 0:
                        state["o_ps"] = opsum.tile(
                            [HD + 1, NQB], F32, tag="o", name="o_ps"
                        )
                    o_ps = state["o_ps"]
                    for a in range(3):
                        nc.tensor.matmul(
                            o_ps[:, :qln],
                            vhat[:, 3 * g + a, vh],
                            ex[:, NQB * a : NQB * a + qln],
                            start=(g == 0 and a == 0),
                            stop=(g == NG - 1 and a == 2),
                        )
                    if g == NG - 1:
                        # block tail: copy out, reciprocal of the ones-row
                        # denominator, broadcast via a DRAM bounce, norm.
                        nc.vector.tensor_copy(
                            out=oaug_h[h][:, qoff : qoff + qln],
                            in_=o_ps[:, :qln],
                        )
                        co = h * N + qoff
                        nc.vector.reciprocal(
                            out=inv_row[HD : HD + 1, co : co + qln],
                            in_=oaug_h[h][HD : HD + 1, qoff : qoff + qln],
                        )
                        nc.sync.dma_start(
                            out=inv_dram[h : h + 1, qoff : qoff + qln],
                            in_=inv_row[HD : HD + 1, co : co + qln],
                        )
                        src = inv_dram[h : h + 1, qoff : qoff + qln]
                        bc = bass.AP(
                            tensor=src.tensor,
                            offset=src.offset,
                            ap=[[0, HD]] + [list(d) for d in src.ap[1:]],
                        )
                        nc.sync.dma_start(
                            out=inv_bc[h][:, qoff : qoff + qln], in_=bc
                        )
                        nc.vector.tensor_mul(
                            out=on_h[h][:, qoff : qoff + qln],
                            in0=oaug_h[h][:HD, qoff : qoff + qln],
                            in1=inv_bc[h][:, qoff : qoff + qln],
                        )

                def emit_wo(qoff, qln):
                    for j in range(qoff // 128, (qoff + qln) // 128):
                        wp = wopsum.tile([128, C], F32, tag="wo", name="wp")
                        for h in range(HPC):
                            nc.tensor.matmul(
                                wp,
                                on_h[h][:, 128 * j : 128 * (j + 1)],
                                wo_h[h],
                                start=(h == 0),
                                stop=(h == HPC - 1),
                            )
                        ot = outp.tile([128, C], F32, tag="ot")
                        nc.vector.tensor_add(out=ot, in0=wp, in1=bo_rep)
                        nc.sync.dma_start(
                            out=out_d[128 * j : 128 * (j + 1), :], in_=ot
                        )

                pend = None
                for idx, (h, qoff, qln, g) in enumerate(stage_list):
                    s_tri = spsum.tile([128, 3 * NQB], F32, tag="s")
                    for a in range(3):
                        nc.tensor.matmul(
                            s_tri[:, NQB * a : NQB * a + qln],
                            k_rep[h][
                                32 * a : 32 * a + 32,
                                384 * g + 128 * a : 384 * g + 128 * a + 128,
                            ],
                            q_rep[h][32 * a : 32 * a + 32, qoff : qoff + qln],
                            start=True,
                            stop=True,
                        )
                    ex = expp.tile([128, 3 * NQB], F32R, tag="ex")
                    nc.scalar.activation(
                        out=ex.rearrange("p (a c) -> p a c", a=3)[:, :, :qln],
                        in_=s_tri.rearrange("p (a c) -> p a c", a=3)[
                            :, :, :qln
                        ],
                        func=mybir.ActivationFunctionType.Exp,
                        scale=SCALE,
                    )
                    flush_pv(pend)
                    pend = (ex, h, qoff, qln, g)
                    # one deferred projection piece every other stage
                    if projq and idx % 2 == 0:
                        name, ph, off, ln = projq.popleft()
                        emit_qk_proj(wopsum, name, ph, off, ln, tag="wo")
                    # Wo for query block b, due once head1's block-b tail
                    # chain has drained: emitted at (h1, block b+1, g==3)
                    if h == 1 and g == 3:
                        bi = qpieces.index((qoff, qln))
                        if bi >= 1:
                            emit_wo(*qpieces[bi - 1])
                flush_pv(pend)
                emit_wo(*qpieces[-1])

    if split:
        _split_excess_waits(nc)
    return nc


def kernel(x, Wq, bq, Wk, bk, Wv, bv, Wo, bo):
    global LAST_RESULTS, _CACHED_NC
    x = np.ascontiguousarray(np.asarray(x, dtype=np.float32))
    Wq = np.asarray(Wq, dtype=np.float32)
    Wk = np.asarray(Wk, dtype=np.float32)
    Wv = np.asarray(Wv, dtype=np.float32)
    Wo = np.asarray(Wo, dtype=np.float32)
    bq = np.asarray(bq, dtype=np.float32)
    bk = np.asarray(bk, dtype=np.float32)
    bv = np.asarray(bv, dtype=np.float32)
    bo = np.asarray(bo, dtype=np.float32)

    def wrep(W, g):
        # [C, 2, 96]: head h cols = W[64g+32h : 64g+32h+32, :].T tiled 3x
        out = np.empty((C, HPC, 96), np.float32)
        for h in range(HPC):
            blk = W[GD * g + HD * h : GD * g + HD * (h + 1), :].T  # [C, 32]
            out[:, h, :] = np.tile(blk, (1, 3))
        return np.ascontiguousarray(out)

    def brep(bvec, g):
        out = np.empty((HPC, 96, 1), np.float32)
        for h in range(HPC):
            out[h, :, 0] = np.tile(bvec[GD * g + HD * h : GD * g + HD * (h + 1)], 3)
        return out

    xf = x.reshape(B, C, N)
    in_maps = []
    for core in range(NC):
        b = core // 4
        g = core % 4
        sl = slice(GD * g, GD * (g + 1))
        in_maps.append(
            {
                "x": np.ascontiguousarray(xf[b]),
                "wq_t": wrep(Wq, g),
                "wk_t": wrep(Wk, g),
                "wv_t": np.ascontiguousarray(Wv[sl, :].T),
                "bq": brep(bq, g),
                "bk": brep(bk, g),
                "bv": np.ascontiguousarray(bv[sl].reshape(GD, 1)),
                "wo_t": np.ascontiguousarray(Wo[:, sl].T),
                "bo": np.ascontiguousarray(
                    bo.reshape(C, 1) if g == 0 else np.zeros((C, 1), np.float32)
                ),
            }
        )

    if _CACHED_NC is None:
        _CACHED_NC = build_nc()
    res = run_bass_kernel_spmd(_CACHED_NC, in_maps, core_ids=list(range(NC)))
    LAST_RESULTS = res

    out = np.zeros((B, C, N), dtype=np.float32)
    for core in range(NC):
        out[core // 4] += res.results[core]["out_t"].T
    return out.reshape(B, C, 48, 48)
